# revision 43
# baseline (speedup 1.0000x reference)
"""Distributed multi-head attention for TRN2 (8 NeuronCores).

Reference computation (per batch b):
    qkv = x @ w_qkv.T                         # (N, 3C)
    q, k, v = split/reshape to (H, N, D)
    attn = softmax(q @ k.T * D**-0.5)         # per head
    out = (attn @ v) reassembled to (N, C)
    out = out @ w_proj.T + b_proj

Sharding: 8 cores = 4 batches x 2 query-halves. Each core computes k/v
for all 2048 tokens of its batch (duplicated across the 2 cores of a
batch - cheaper than communicating), q for its own 1024 tokens, the
full attention for all 12 heads over its 1024 queries, and the output
projection. No collectives.

Layout strategy (all chosen so no on-chip transposes are needed):
  - host passes x^T and w_qkv^T so projections contract over partitions
  - q,k are produced "d-major" ([head-dim, tokens]) via out^T-form
    matmuls; scores are computed transposed ([keys, queries]) which is
    exactly the layout attn@v consumes as its stationary-side operand
  - softmax needs no max-subtraction (scores ~ N(0,1), fp32 exp range)
  - the denominator rides along as a ones-column appended to v (M=65
    matmuls); normalization uses a K=1 ones-matmul to broadcast 1/denom
    across partitions
  - all matmuls in bf16 (PSUM accumulation is fp32); softmax exp runs
    on the scalar (ACT) engine from PSUM f32, writing bf16 probs

Schedule: the ACT engine (softmax exp, ~1us per 128x1024 tile) is the
steady-state bottleneck; everything else hides under it. Attention runs
as 12 passes (head pair x query half). Per pass and k-block: the two
heads' score matmuls write one shared PSUM tile, alternating PE row
groups (base partition 0/64) so they run concurrently; exp(kb) overlaps
scores(kb+1) via two PSUM slots; attn@v lags by one k-block. The query
halving keeps the pass's PSUM footprint at 6 banks, leaving 2 banks for
"filler" projection work that keeps the PE busy (and its HAM clock
warm): pass 0 produces v block kb just-in-time in step kb, passes 1-5
drain the k/q blocks of later pairs. The per-pass normalization
epilogue is split so its PE part lands inside the next pass.

Self-contained: hardcodes B=4, N=2048, C=768, H=12, D=64.
"""

import numpy as np
import ml_dtypes

import concourse.bass as bass
import concourse.mybir as mybir
from concourse import bacc
from concourse.tile import TileContext
from concourse.bass_utils import run_bass_kernel_spmd

F32 = mybir.dt.float32
BF16 = mybir.dt.bfloat16
EXP = mybir.ActivationFunctionType.Exp

B, N, C = 4, 2048, 768
H, D = 12, 64
SCALE = float(D) ** -0.5  # 0.125
NQ = N // 2  # queries per core: 1024
CB = C // 128  # 6 c-chunks
TB = N // 128  # 16 token blocks
HB = H // 2  # 6 head pairs
VW = H * (D + 1)  # 780: v block width with ones columns

N_CORES = 8

# w_qkv columns, grouped in the order the projection units consume them:
# pair-0 k/q, v in three 256-col thirds, then k/q for pairs 1..5. Each
# group holds its column range for all six 128-row input chunks,
# contiguously.
_WQ_GROUPS = [(C, 128), (0, 128)]
for _vo in range(3):
    _WQ_GROUPS.append((2 * C + _vo * 256, 256))
for _ob in range(1, CB):
    _WQ_GROUPS.append((C + _ob * 128, 128))
    _WQ_GROUPS.append((_ob * 128, 128))
_WQ_BASE = {}
_cur = 0
for _o0, _w in _WQ_GROUPS:
    _WQ_BASE[_o0] = (_cur, _w)
    _cur += CB * _w


def _build():
    nc = bacc.Bacc(None, target_bir_lowering=False)

    # host-packed SBUF images: xTp cols = [tch][ci][t]; wqp cols grouped
    # in consumption order (see _WQ_GROUPS)
    xTp = nc.declare_dram_parameter("xTp", [128, CB * N], BF16, isOutput=False)
    wqp = nc.declare_dram_parameter("wqp", [128, CB * 3 * C], BF16, isOutput=False)
    wprojp = nc.declare_dram_parameter("wprojp", [128, CB * C], BF16, isOutput=False)
    biasp = nc.declare_dram_parameter("biasp", [128, CB], F32, isOutput=False)
    outT = nc.declare_dram_parameter("outT", [C, NQ], BF16, isOutput=True)

    with TileContext(nc) as tc:
        with (
            tc.tile_pool(name="per", bufs=1) as per,
            tc.tile_pool(name="p23", bufs=1) as p23,
            tc.tile_pool(name="hp", bufs=8) as hp,
            tc.tile_pool(name="mi", bufs=3) as mi,
            tc.tile_pool(name="op", bufs=2) as op_pool,
            tc.tile_pool(name="ps", bufs=2, space="PSUM") as ps2,
        ):
            # ---- persistent tiles -------------------------------------
            qT_sb = per.tile([128, CB * NQ], BF16)  # q^T  [2 heads/blk, 1024]
            kT_sb = per.tile([128, CB * N], BF16)  # k^T  [2 heads/blk, 2048]
            vaug_sb = per.tile([128, TB * VW], BF16)  # v + ones cols
            bias_sb = per.tile([128, CB], F32)
            ones_sb = per.tile([1, 64], BF16)
            attnT_sb = p23.tile([128, CB * NQ], BF16)  # attn out^T
            wproj_sb = p23.tile([128, CB * C], BF16)

            nc.vector.memset(ones_sb[:, :], 1.0)
            # ones columns of vaug: col 64 of each 65-wide head slot
            vaug_ones = vaug_sb[:, :].rearrange(
                "p (t h x) -> p t h x", t=TB, h=H, x=D + 1
            )[:, :, :, D : D + 1]
            nc.vector.memset(vaug_ones, 1.0)

            # weights + activations pools, closed once the projection
            # filler has consumed them
            wqxt = (tc.tile_pool(name="wq", bufs=1), tc.tile_pool(name="xt", bufs=4))
            wq_pool = wqxt[0].__enter__()
            xt_pool = wqxt[1].__enter__()

            wqkv_sb = wq_pool.tile([128, CB * 3 * C], BF16)
            xt0a = xt_pool.tile([128, 3 * 512], BF16, tag="xta", name="xt0a")
            xt0b = xt_pool.tile([128, 3 * 512], BF16, tag="xtb", name="xt0b")
            xts = [None] + [
                xt_pool.tile([128, CB * 512], BF16, tag="xt", name=f"xt{t}")
                for t in range(1, 4)
            ]

            def xtsl(tch, ci, o0, w):
                # column slice [o0, o0+w) of token-chunk tch, c-chunk ci
                if tch == 0:
                    t_ = xt0a if ci < 3 else xt0b
                    c_ = ci % 3
                else:
                    t_ = xts[tch]
                    c_ = ci
                return t_[:, c_ * 512 + o0 : c_ * 512 + o0 + w]

            def _dma_xt(tch):
                nc.sync.dma_start(
                    out=xts[tch][:, :],
                    in_=xTp[:, tch * CB * 512 : (tch + 1) * CB * 512],
                )

            def _dma_wq(gi):
                o0, w = _WQ_GROUPS[gi]
                base, _ = _WQ_BASE[o0]
                nc.sync.dma_start(
                    out=wqkv_sb[:, base : base + CB * w],
                    in_=wqp[:, base : base + CB * w],
                )

            # consumption order: pair-0 k/q cols and x chunk 0 (halved so
            # the first kq unit starts after half the transfer) + first v
            # third (pass-0 JIT), then the remaining token chunks, the
            # other v thirds, later pairs' k/q cols
            nc.sync.dma_start(
                out=wqkv_sb[:, : 2 * CB * 128], in_=wqp[:, : 2 * CB * 128]
            )
            nc.sync.dma_start(out=xt0a[:, :], in_=xTp[:, : CB * 256])
            nc.sync.dma_start(
                out=xt0b[:, :], in_=xTp[:, CB * 256 : CB * 512]
            )
            _dma_wq(2)
            for t in range(1, 4):
                _dma_xt(t)
            for gi in range(3, len(_WQ_GROUPS)):
                _dma_wq(gi)

            def wq(ci, o0, width):
                if o0 >= 2 * C:
                    vg = (o0 - 2 * C) // 256
                    base, gw = _WQ_BASE[2 * C + vg * 256]
                    off = (o0 - 2 * C) % 256
                    assert off + width <= 256
                else:
                    base, gw = _WQ_BASE[o0]
                    off = 0
                return wqkv_sb[:, base + ci * gw + off : base + ci * gw + off + width]

            # phase-2/3-only weights: after the critical-path DMAs
            nc.sync.dma_start(out=bias_sb[:, :], in_=biasp[:, :])
            nc.sync.dma_start(out=wproj_sb[:, :], in_=wprojp[:, :])

            # ---- projection work units (PE filler) --------------------
            def kq_unit(ob, tch, is_q):
                """one k^T (or q^T) block: out-dims block ob, 512 tokens"""
                t0 = tch * 512
                kind = "q" if is_q else "k"
                psv = ps2.tile(
                    [128, 512], F32, tag="psV", bufs=2, name=f"{kind}{ob}_{tch}"
                )
                for ci in range(CB):
                    nc.tensor.matmul(
                        psv[:, :],
                        wq(ci, (0 if is_q else C) + ob * 128, 128),
                        xtsl(tch, ci, 0, 512),
                        start=(ci == 0),
                        stop=(ci == CB - 1),
                    )
                if is_q:
                    nc.vector.tensor_copy(
                        qT_sb[:, ob * NQ + t0 : ob * NQ + t0 + 512], psv[:, :]
                    )
                else:
                    nc.vector.tensor_copy(
                        kT_sb[:, ob * N + t0 : ob * N + t0 + 512], psv[:, :]
                    )

            def v_unit(t128, o0, w):
                """one v unit: 128 tokens x [o0, o0+w) v-dims, written
                (bf16) into the vaug slot layout"""
                tch, tb = divmod(t128, 4)
                psv = ps2.tile(
                    [128, 512], F32, tag="psV", bufs=2, name=f"v{t128}_{o0}"
                )
                for ci in range(CB):
                    nc.tensor.matmul(
                        psv[:, :w],
                        xtsl(tch, ci, tb * 128, 128),
                        wq(ci, 2 * C + o0, w),
                        start=(ci == 0),
                        stop=(ci == CB - 1),
                    )
                nh = w // D
                src = psv[:, :w].rearrange("p (h x) -> p h x", x=D)
                h0 = o0 // D
                base = t128 * VW + h0 * (D + 1)
                dst = vaug_sb[:, base : base + nh * (D + 1)].rearrange(
                    "p (h x) -> p h x", x=D + 1
                )[:, :, :D]
                nc.vector.tensor_copy(dst, src)

            # deferred projection units, drained by the pass fillers in
            # order; per-pass quotas keep every resource complete before
            # its first consumer pass (ob p / v pair p by pass 2p)
            genq = []
            def _push_kq(ob):
                for tch in range(4):
                    genq.append(("kq", ob, tch, False))
                for tch in range(2):
                    genq.append(("kq", ob, tch, True))
            _push_kq(1)
            _push_kq(2)
            for kb in range(TB):
                genq.append(("v", kb, 256, 256))
            _push_kq(3)
            _push_kq(4)
            for kb in range(TB):
                genq.append(("v", kb, 512, 256))
            _push_kq(5)

            def fill_gen():
                if genq:
                    u = genq.pop(0)
                    if u[0] == "kq":
                        kq_unit(u[1], u[2], u[3])
                    else:
                        v_unit(u[1], u[2], u[3])

            # ---- attention machinery ----------------------------------
            def epi_pe(hb_, qc_, outs_):
                """PE part of a pass's normalization epilogue. The two
                heads' 1/denom broadcasts go to different column strips of
                one PSUM tile (col tiling) so they run concurrently."""
                psb = ps2.tile(
                    [128, 512], F32, tag="psV", bufs=2,
                    name=f"psb{hb_}_{qc_}",
                )
                for hh_ in range(2):
                    nc.tensor.matmul(
                        psb[64 * hh_ : 64 * hh_ + 64, :],
                        ones_sb[:, :],
                        outs_[hh_][1][:, :],
                        start=True,
                        stop=True,
                    )
                for hh_ in range(2):
                    nc.vector.tensor_mul(
                        attnT_sb[
                            64 * hh_ : 64 * hh_ + 64,
                            hb_ * NQ + qc_ * 512 : hb_ * NQ + (qc_ + 1) * 512,
                        ],
                        psb[64 * hh_ : 64 * hh_ + 64, :],
                        outs_[hh_][0][:, :],
                    )

            def emit_pass(hb, qc, pend, filler=None):
                """One (head pair, query half) attention pass."""
                q0 = hb * NQ + qc * 512
                accs = [
                    ps2.tile(
                        [128, 512], F32, tag="psA", bufs=2,
                        name=f"acc{hb}_{qc}_{i}",
                    )
                    for i in range(2)
                ]
                def av_mms(pkb, ppb):
                    for hh in range(2):
                        vs = pkb * VW + (2 * hb + hh) * (D + 1)
                        nc.tensor.matmul(
                            accs[hh][0:65, :],
                            vaug_sb[:, vs : vs + D + 1],
                            ppb[:, hh * 512 : (hh + 1) * 512],
                            start=(pkb == 0),
                            stop=(pkb == TB - 1),
                        )

                # two k-blocks per step: the 4 score matmuls form an
                # alternating row-group run so their weight loads pipeline
                prev = []
                pend_outs = None
                for kb2 in range(0, TB, 2):
                    scs = []
                    for kb in (kb2, kb2 + 1):
                        sc = ps2.tile(
                            [128, NQ], F32, tag="psS", bufs=2,
                            name=f"sc{hb}_{qc}_{kb}",
                        )
                        for hh in range(2):
                            p0 = 64 * hh
                            nc.tensor.matmul(
                                sc[:, hh * 512 : (hh + 1) * 512],
                                kT_sb[
                                    p0 : p0 + 64,
                                    hb * N + kb * 128 : hb * N + (kb + 1) * 128,
                                ],
                                qT_sb[p0 : p0 + 64, q0 : q0 + 512],
                                start=True,
                                stop=True,
                                tile_position=(p0, 0),
                            )
                        scs.append(sc)
                    if filler is not None:
                        filler(kb2)
                        filler(kb2 + 1)
                    if kb2 == 0 and pend is not None:
                        # the previous pass's last attn@v pair + DVE
                        # epilogue run here, AFTER this pass's first
                        # scores, so the final exp of pass p never blocks
                        # exp(p+1, kb0) through the PE queue
                        pend_outs = pend[2]()
                    for pkb, ppb in prev:
                        av_mms(pkb, ppb)
                    prev = []
                    for i, kb in enumerate((kb2, kb2 + 1)):
                        pb = hp.tile([128, NQ], BF16, tag="probs")
                        nc.scalar.activation(
                            pb[:, :], scs[i][:, :], EXP, scale=SCALE
                        )
                        prev.append((kb, pb))
                    if kb2 == 4 and pend is not None:
                        epi_pe(pend[0], pend[1], pend_outs)
                        pend = None

                def finish():
                    # drain attn@v for the last two k-blocks, then the
                    # DVE accumulator/denominator epilogue
                    for pkb, ppb in prev:
                        av_mms(pkb, ppb)
                    outs = []
                    for hh in range(2):
                        acc = accs[hh]
                        cpy = mi.tile([64, 512], F32, tag="cpy")
                        nc.vector.tensor_copy(cpy[:, :], acc[0:64, :])
                        den = mi.tile([1, 512], F32, tag="den")
                        nc.vector.tensor_copy(den[:, :], acc[64:65, :])
                        rec = mi.tile([1, 512], F32, tag="rec")
                        nc.vector.reciprocal_approx_fast(rec[:, :], den[:, :])
                        row = mi.tile([1, 512], BF16, tag="row")
                        nc.vector.tensor_copy(row[:, :], rec[:, :])
                        outs.append((cpy, row))
                    return outs

                return (hb, qc, finish)

            # ---- pre-phase ---------------------------------------------
            # dummy matmuls on the ones tile: ~3us of PE activity flips
            # the HAM clock gate to 8/8 before the first real unit, which
            # would otherwise run at 1.2 GHz while waiting out the window
            warm = ps2.tile([128, 512], F32, tag="psV", bufs=2, name="warm")
            for wi in range(22):
                nc.tensor.matmul(
                    warm[0:64, 0:64],
                    ones_sb[:, :],
                    ones_sb[:, :],
                    start=(wi == 0),
                    stop=(wi == 21),
                )
            # k/q ob0 for the first tokens
            kq_unit(0, 0, False)
            kq_unit(0, 0, True)

            # ---- phase 2: 12 passes -----------------------------------
            # pass 0 produces v pairs 0-1 just-in-time (block kb in step
            # kb, one step before attn@v needs it) and finishes pair-0
            # k/q as its DMAs land; later passes drain genq under quotas
            _P0_SPECIAL = {
                2: (0, 1, False),
                5: (0, 2, False),
                7: (0, 3, False),
                9: (0, 1, True),
            }

            def fill_p0(kb):
                v_unit(kb, 0, 256)
                if kb in _P0_SPECIAL:
                    kq_unit(*_P0_SPECIAL[kb])

            _QUOTA = [0, 8, 10, 11, 7, 7, 7, 7, 3, 2, 2, 0]

            def mk_fill(pi):
                quota = _QUOTA[pi]
                if quota == 0:
                    return None
                done = [0]

                def fill(kb):
                    want = (kb + 1) * quota // TB
                    while done[0] < want:
                        fill_gen()
                        done[0] += 1

                return fill

            # ---- phase 3: output projection (out^T form), emitted as
            # (ob, query-half) units so they pipeline through PSUM/DVE/DMA;
            # the qc=0 units for ob 0-3 run as pass-11 fillers (their last
            # attnT column block lands at pass 11's start via the deferred
            # epilogue)
            COPYFN = mybir.ActivationFunctionType.Identity

            def proj_unit(ob, qc, use_act=False):
                psp = ps2.tile(
                    [128, 512], F32, tag="psV", bufs=2, name=f"prj{ob}_{qc}"
                )
                for cb in range(CB):
                    nc.tensor.matmul(
                        psp[:, :],
                        wproj_sb[:, cb * C + ob * 128 : cb * C + (ob + 1) * 128],
                        attnT_sb[:, cb * NQ + qc * 512 : cb * NQ + (qc + 1) * 512],
                        start=(cb == 0),
                        stop=(cb == CB - 1),
                    )
                if qc == 0:
                    _ot_tiles[ob] = op_pool.tile(
                        [128, NQ], BF16, tag="out", bufs=6, name=f"ot{ob}"
                    )
                ot = _ot_tiles[ob]
                nc.vector.tensor_scalar_add(
                    ot[:, qc * 512 : (qc + 1) * 512],
                    psp[:, :],
                    bias_sb[:, ob : ob + 1],
                )
                nc.sync.dma_start(
                    out=outT[ob * 128 : (ob + 1) * 128, qc * 512 : (qc + 1) * 512],
                    in_=ot[:, qc * 512 : (qc + 1) * 512],
                )

            _ot_tiles = {}
            _P11_PROJ = {6: 0, 10: 1, 14: 2}

            def fill_p11(kb):
                if kb in _P11_PROJ:
                    proj_unit(_P11_PROJ[kb], 0)

            pend = emit_pass(0, 0, None, filler=fill_p0)
            for pi in range(1, 2 * HB):
                hb, qc = divmod(pi, 2)
                filler = fill_p11 if pi == 11 else mk_fill(pi)
                pend = emit_pass(hb, qc, pend, filler=filler)
            outs_last = pend[2]()
            for ob in range(3, CB):
                proj_unit(ob, 0, use_act=True)
            # qc=1 projection: only the cb5 term depends on the final
            # epilogue, so accumulate cb0-4 for all six obs first (the
            # retired score ring + accumulator banks hold the partials),
            # then emit the epilogue and finish each ob with one matmul
            q1p = {}
            bigs = [
                ps2.tile([128, NQ], F32, tag="psS", bufs=2, name=f"q1s{i}")
                for i in range(2)
            ]
            for ob in range(4):
                q1p[ob] = bigs[ob // 2][:, (ob % 2) * 512 : (ob % 2) * 512 + 512]
            for ob in (4, 5):
                q1p[ob] = ps2.tile(
                    [128, 512], F32, tag="psA", bufs=2, name=f"q1a{ob}"
                )[:, :]
            for ob in range(CB):
                for cb in range(CB - 1):
                    nc.tensor.matmul(
                        q1p[ob],
                        wproj_sb[:, cb * C + ob * 128 : cb * C + (ob + 1) * 128],
                        attnT_sb[:, cb * NQ + 512 : cb * NQ + NQ],
                        start=(cb == 0),
                        stop=False,
                    )
            epi_pe(pend[0], pend[1], outs_last)
            assert not genq

            wqxt[1].__exit__(None, None, None)
            wqxt[0].__exit__(None, None, None)

            for ob in range(CB):
                nc.tensor.matmul(
                    q1p[ob],
                    wproj_sb[:, 5 * C + ob * 128 : 5 * C + (ob + 1) * 128],
                    attnT_sb[:, 5 * NQ + 512 : 5 * NQ + NQ],
                    start=False,
                    stop=True,
                )
                ot = _ot_tiles[ob]
                nc.vector.tensor_scalar_add(
                    ot[:, 512:NQ], q1p[ob], bias_sb[:, ob : ob + 1]
                )
                nc.sync.dma_start(
                    out=outT[ob * 128 : (ob + 1) * 128, 512:NQ],
                    in_=ot[:, 512:NQ],
                )

    nc.finalize()
    return nc


_NC_CACHE = []


def _get_nc():
    if not _NC_CACHE:
        _NC_CACHE.append(_build())
    return _NC_CACHE[0]


def kernel(x, w_qkv, w_proj, b_proj):
    x = np.asarray(x, dtype=np.float32)
    w_qkv = np.asarray(w_qkv, dtype=np.float32)
    w_proj = np.asarray(w_proj, dtype=np.float32)
    b_proj = np.asarray(b_proj, dtype=np.float32)

    nc = _get_nc()

    wqkvT = w_qkv.T.astype(ml_dtypes.bfloat16)  # [C, 3C]
    wq3 = np.ascontiguousarray(wqkvT).reshape(CB, 128, 3 * C)  # [ci, p, o]
    wqp = np.concatenate(
        [
            wq3[:, :, o0 : o0 + w].transpose(1, 0, 2).reshape(128, CB * w)
            for o0, w in _WQ_GROUPS
        ],
        axis=1,
    )
    wqp = np.ascontiguousarray(wqp)
    # SBUF images: wproj cols = [ci][o], bias cols = [ci]
    wprojp = np.ascontiguousarray(
        w_proj.T.astype(ml_dtypes.bfloat16).reshape(CB, 128, C)
        .transpose(1, 0, 2)
        .reshape(128, CB * C)
    )
    biasp = np.ascontiguousarray(
        b_proj.astype(np.float32).reshape(CB, 128).T
    )

    in_maps = []
    for core in range(N_CORES):
        b, half = divmod(core, 2)
        # own 1024 query tokens first, then the other half (key order
        # within attention is permutation-invariant)
        mine = x[b, half * NQ : (half + 1) * NQ].T
        other = x[b, (1 - half) * NQ : (2 - half) * NQ].T
        xTc = np.concatenate([mine, other], axis=1).astype(ml_dtypes.bfloat16)
        # pack to the SBUF image: cols = [tch][ci][t]
        xTp = np.ascontiguousarray(
            xTc.reshape(CB, 128, 4, 512).transpose(1, 2, 0, 3).reshape(128, CB * N)
        )
        in_maps.append({"xTp": xTp, "wqp": wqp, "wprojp": wprojp, "biasp": biasp})

    res = run_bass_kernel_spmd(nc, in_maps, core_ids=list(range(N_CORES)))

    out = np.empty((B, N, C), dtype=np.float32)
    for core in range(N_CORES):
        b, half = divmod(core, 2)
        out[b, half * NQ : (half + 1) * NQ, :] = (
            res.results[core]["outT"].astype(np.float32).T
        )
    return out



# revision 44
# speedup vs baseline: 1.0013x; 1.0013x over previous
"""Distributed multi-head attention for TRN2 (8 NeuronCores).

Reference computation (per batch b):
    qkv = x @ w_qkv.T                         # (N, 3C)
    q, k, v = split/reshape to (H, N, D)
    attn = softmax(q @ k.T * D**-0.5)         # per head
    out = (attn @ v) reassembled to (N, C)
    out = out @ w_proj.T + b_proj

Sharding: 8 cores = 4 batches x 2 query-halves. Each core computes k/v
for all 2048 tokens of its batch (duplicated across the 2 cores of a
batch - cheaper than communicating), q for its own 1024 tokens, the
full attention for all 12 heads over its 1024 queries, and the output
projection. No collectives.

Layout strategy (all chosen so no on-chip transposes are needed):
  - host passes x^T and w_qkv^T so projections contract over partitions
  - q,k are produced "d-major" ([head-dim, tokens]) via out^T-form
    matmuls; scores are computed transposed ([keys, queries]) which is
    exactly the layout attn@v consumes as its stationary-side operand
  - softmax needs no max-subtraction (scores ~ N(0,1), fp32 exp range)
  - the denominator rides along as a ones-column appended to v (M=65
    matmuls); normalization uses a K=1 ones-matmul to broadcast 1/denom
    across partitions
  - all matmuls in bf16 (PSUM accumulation is fp32); softmax exp runs
    on the scalar (ACT) engine from PSUM f32, writing bf16 probs

Schedule: the ACT engine (softmax exp, ~1us per 128x1024 tile) is the
steady-state bottleneck; everything else hides under it. Attention runs
as 12 passes (head pair x query half). Per pass and k-block: the two
heads' score matmuls write one shared PSUM tile, alternating PE row
groups (base partition 0/64) so they run concurrently; exp(kb) overlaps
scores(kb+1) via two PSUM slots; attn@v lags by one k-block. The query
halving keeps the pass's PSUM footprint at 6 banks, leaving 2 banks for
"filler" projection work that keeps the PE busy (and its HAM clock
warm): pass 0 produces v block kb just-in-time in step kb, passes 1-5
drain the k/q blocks of later pairs. The per-pass normalization
epilogue is split so its PE part lands inside the next pass.

Self-contained: hardcodes B=4, N=2048, C=768, H=12, D=64.
"""

import numpy as np
import ml_dtypes

import concourse.bass as bass
import concourse.mybir as mybir
from concourse import bacc
from concourse.tile import TileContext
from concourse.bass_utils import run_bass_kernel_spmd

F32 = mybir.dt.float32
BF16 = mybir.dt.bfloat16
EXP = mybir.ActivationFunctionType.Exp

B, N, C = 4, 2048, 768
H, D = 12, 64
SCALE = float(D) ** -0.5  # 0.125
NQ = N // 2  # queries per core: 1024
CB = C // 128  # 6 c-chunks
TB = N // 128  # 16 token blocks
HB = H // 2  # 6 head pairs
VW = H * (D + 1)  # 780: v block width with ones columns

N_CORES = 8

# w_qkv columns, grouped in the order the projection units consume them:
# pair-0 k/q, v in three 256-col thirds, then k/q for pairs 1..5. Each
# group holds its column range for all six 128-row input chunks,
# contiguously.
_WQ_GROUPS = [(C, 128), (0, 128)]
for _vo in range(3):
    _WQ_GROUPS.append((2 * C + _vo * 256, 256))
for _ob in range(1, CB):
    _WQ_GROUPS.append((C + _ob * 128, 128))
    _WQ_GROUPS.append((_ob * 128, 128))
_WQ_BASE = {}
_cur = 0
for _o0, _w in _WQ_GROUPS:
    _WQ_BASE[_o0] = (_cur, _w)
    _cur += CB * _w


def _build():
    nc = bacc.Bacc(None, target_bir_lowering=False)

    # host-packed SBUF images: xTp cols = [tch][ci][t]; wqp cols grouped
    # in consumption order (see _WQ_GROUPS)
    xTp = nc.declare_dram_parameter("xTp", [128, CB * N], BF16, isOutput=False)
    wqp = nc.declare_dram_parameter("wqp", [128, CB * 3 * C], BF16, isOutput=False)
    wprojp = nc.declare_dram_parameter("wprojp", [128, CB * C], BF16, isOutput=False)
    biasp = nc.declare_dram_parameter("biasp", [128, CB], F32, isOutput=False)
    outT = nc.declare_dram_parameter("outT", [C, NQ], BF16, isOutput=True)

    with TileContext(nc) as tc:
        with (
            tc.tile_pool(name="per", bufs=1) as per,
            tc.tile_pool(name="p23", bufs=1) as p23,
            tc.tile_pool(name="hp", bufs=8) as hp,
            tc.tile_pool(name="mi", bufs=3) as mi,
            tc.tile_pool(name="op", bufs=2) as op_pool,
            tc.tile_pool(name="ps", bufs=2, space="PSUM") as ps2,
        ):
            # ---- persistent tiles -------------------------------------
            qT_sb = per.tile([128, CB * NQ], BF16)  # q^T  [2 heads/blk, 1024]
            kT_sb = per.tile([128, CB * N], BF16)  # k^T  [2 heads/blk, 2048]
            vaug_sb = per.tile([128, TB * VW], BF16)  # v + ones cols
            bias_sb = per.tile([128, CB], F32)
            ones_sb = per.tile([1, 64], BF16)
            attnT_sb = p23.tile([128, CB * NQ], BF16)  # attn out^T
            wproj_sb = p23.tile([128, CB * C], BF16)

            nc.vector.memset(ones_sb[:, :], 1.0)
            # ones columns of vaug: col 64 of each 65-wide head slot
            vaug_ones = vaug_sb[:, :].rearrange(
                "p (t h x) -> p t h x", t=TB, h=H, x=D + 1
            )[:, :, :, D : D + 1]
            nc.vector.memset(vaug_ones, 1.0)

            # weights + activations pools, closed once the projection
            # filler has consumed them
            wqxt = (tc.tile_pool(name="wq", bufs=1), tc.tile_pool(name="xt", bufs=4))
            wq_pool = wqxt[0].__enter__()
            xt_pool = wqxt[1].__enter__()

            wqkv_sb = wq_pool.tile([128, CB * 3 * C], BF16)
            xt0a = xt_pool.tile([128, 3 * 512], BF16, tag="xta", name="xt0a")
            xt0b = xt_pool.tile([128, 3 * 512], BF16, tag="xtb", name="xt0b")
            xts = [None] + [
                xt_pool.tile([128, CB * 512], BF16, tag="xt", name=f"xt{t}")
                for t in range(1, 4)
            ]

            def xtsl(tch, ci, o0, w):
                # column slice [o0, o0+w) of token-chunk tch, c-chunk ci
                if tch == 0:
                    t_ = xt0a if ci < 3 else xt0b
                    c_ = ci % 3
                else:
                    t_ = xts[tch]
                    c_ = ci
                return t_[:, c_ * 512 + o0 : c_ * 512 + o0 + w]

            def _dma_xt(tch):
                nc.sync.dma_start(
                    out=xts[tch][:, :],
                    in_=xTp[:, tch * CB * 512 : (tch + 1) * CB * 512],
                )

            def _dma_wq(gi):
                o0, w = _WQ_GROUPS[gi]
                base, _ = _WQ_BASE[o0]
                nc.sync.dma_start(
                    out=wqkv_sb[:, base : base + CB * w],
                    in_=wqp[:, base : base + CB * w],
                )

            # consumption order: pair-0 k/q cols and x chunk 0 (halved so
            # the first kq unit starts after half the transfer) + first v
            # third (pass-0 JIT), then the remaining token chunks, the
            # other v thirds, later pairs' k/q cols
            nc.sync.dma_start(
                out=wqkv_sb[:, : 2 * CB * 128], in_=wqp[:, : 2 * CB * 128]
            )
            nc.sync.dma_start(out=xt0a[:, :], in_=xTp[:, : CB * 256])
            nc.sync.dma_start(
                out=xt0b[:, :], in_=xTp[:, CB * 256 : CB * 512]
            )
            _dma_wq(2)
            for t in range(1, 4):
                _dma_xt(t)
            for gi in range(3, len(_WQ_GROUPS)):
                _dma_wq(gi)

            def wq(ci, o0, width):
                if o0 >= 2 * C:
                    vg = (o0 - 2 * C) // 256
                    base, gw = _WQ_BASE[2 * C + vg * 256]
                    off = (o0 - 2 * C) % 256
                    assert off + width <= 256
                else:
                    base, gw = _WQ_BASE[o0]
                    off = 0
                return wqkv_sb[:, base + ci * gw + off : base + ci * gw + off + width]

            # phase-2/3-only weights: after the critical-path DMAs
            nc.sync.dma_start(out=bias_sb[:, :], in_=biasp[:, :])
            nc.sync.dma_start(out=wproj_sb[:, :], in_=wprojp[:, :])

            # ---- projection work units (PE filler) --------------------
            def kq_unit(ob, tch, is_q):
                """one k^T (or q^T) block: out-dims block ob, 512 tokens"""
                t0 = tch * 512
                kind = "q" if is_q else "k"
                psv = ps2.tile(
                    [128, 512], F32, tag="psV", bufs=2, name=f"{kind}{ob}_{tch}"
                )
                for ci in range(CB):
                    nc.tensor.matmul(
                        psv[:, :],
                        wq(ci, (0 if is_q else C) + ob * 128, 128),
                        xtsl(tch, ci, 0, 512),
                        start=(ci == 0),
                        stop=(ci == CB - 1),
                    )
                if is_q:
                    nc.vector.tensor_copy(
                        qT_sb[:, ob * NQ + t0 : ob * NQ + t0 + 512], psv[:, :]
                    )
                else:
                    nc.vector.tensor_copy(
                        kT_sb[:, ob * N + t0 : ob * N + t0 + 512], psv[:, :]
                    )

            def v_unit(t128, o0, w):
                """one v unit: 128 tokens x [o0, o0+w) v-dims, written
                (bf16) into the vaug slot layout"""
                tch, tb = divmod(t128, 4)
                psv = ps2.tile(
                    [128, 512], F32, tag="psV", bufs=2, name=f"v{t128}_{o0}"
                )
                for ci in range(CB):
                    nc.tensor.matmul(
                        psv[:, :w],
                        xtsl(tch, ci, tb * 128, 128),
                        wq(ci, 2 * C + o0, w),
                        start=(ci == 0),
                        stop=(ci == CB - 1),
                    )
                nh = w // D
                src = psv[:, :w].rearrange("p (h x) -> p h x", x=D)
                h0 = o0 // D
                base = t128 * VW + h0 * (D + 1)
                dst = vaug_sb[:, base : base + nh * (D + 1)].rearrange(
                    "p (h x) -> p h x", x=D + 1
                )[:, :, :D]
                nc.vector.tensor_copy(dst, src)

            # deferred projection units, drained by the pass fillers in
            # order; per-pass quotas keep every resource complete before
            # its first consumer pass (ob p / v pair p by pass 2p)
            genq = []
            def _push_kq(ob):
                for tch in range(4):
                    genq.append(("kq", ob, tch, False))
                for tch in range(2):
                    genq.append(("kq", ob, tch, True))
            _push_kq(1)
            _push_kq(2)
            for kb in range(TB):
                genq.append(("v", kb, 256, 256))
            _push_kq(3)
            _push_kq(4)
            for kb in range(TB):
                genq.append(("v", kb, 512, 256))
            _push_kq(5)

            def fill_gen():
                if genq:
                    u = genq.pop(0)
                    if u[0] == "kq":
                        kq_unit(u[1], u[2], u[3])
                    else:
                        v_unit(u[1], u[2], u[3])

            # ---- attention machinery ----------------------------------
            def epi_pe(hb_, qc_, outs_):
                """PE part of a pass's normalization epilogue. The two
                heads' 1/denom broadcasts go to different column strips of
                one PSUM tile (col tiling) so they run concurrently."""
                psb = ps2.tile(
                    [128, 512], F32, tag="psV", bufs=2,
                    name=f"psb{hb_}_{qc_}",
                )
                for hh_ in range(2):
                    nc.tensor.matmul(
                        psb[64 * hh_ : 64 * hh_ + 64, :],
                        ones_sb[:, :],
                        outs_[hh_][1][:, :],
                        start=True,
                        stop=True,
                    )
                for hh_ in range(2):
                    nc.vector.tensor_mul(
                        attnT_sb[
                            64 * hh_ : 64 * hh_ + 64,
                            hb_ * NQ + qc_ * 512 : hb_ * NQ + (qc_ + 1) * 512,
                        ],
                        psb[64 * hh_ : 64 * hh_ + 64, :],
                        outs_[hh_][0][:, :],
                    )

            def emit_pass(hb, qc, pend, filler=None):
                """One (head pair, query half) attention pass."""
                q0 = hb * NQ + qc * 512
                accs = [
                    ps2.tile(
                        [128, 512], F32, tag="psA", bufs=2,
                        name=f"acc{hb}_{qc}_{i}",
                    )
                    for i in range(2)
                ]
                def av_mms(pkb, ppb):
                    for hh in range(2):
                        vs = pkb * VW + (2 * hb + hh) * (D + 1)
                        nc.tensor.matmul(
                            accs[hh][0:65, :],
                            vaug_sb[:, vs : vs + D + 1],
                            ppb[:, hh * 512 : (hh + 1) * 512],
                            start=(pkb == 0),
                            stop=(pkb == TB - 1),
                        )

                # two k-blocks per step: the 4 score matmuls form an
                # alternating row-group run so their weight loads pipeline
                prev = []
                pend_outs = None
                for kb2 in range(0, TB, 2):
                    scs = []
                    for kb in (kb2, kb2 + 1):
                        sc = ps2.tile(
                            [128, NQ], F32, tag="psS", bufs=2,
                            name=f"sc{hb}_{qc}_{kb}",
                        )
                        for hh in range(2):
                            p0 = 64 * hh
                            nc.tensor.matmul(
                                sc[:, hh * 512 : (hh + 1) * 512],
                                kT_sb[
                                    p0 : p0 + 64,
                                    hb * N + kb * 128 : hb * N + (kb + 1) * 128,
                                ],
                                qT_sb[p0 : p0 + 64, q0 : q0 + 512],
                                start=True,
                                stop=True,
                                tile_position=(p0, 0),
                            )
                        scs.append(sc)
                    if filler is not None:
                        filler(kb2)
                        filler(kb2 + 1)
                    if kb2 == 0 and pend is not None:
                        # the previous pass's last attn@v pair + DVE
                        # epilogue run here, AFTER this pass's first
                        # scores, so the final exp of pass p never blocks
                        # exp(p+1, kb0) through the PE queue
                        pend_outs = pend[2]()
                    for pkb, ppb in prev:
                        av_mms(pkb, ppb)
                    prev = []
                    for i, kb in enumerate((kb2, kb2 + 1)):
                        pb = hp.tile([128, NQ], BF16, tag="probs")
                        nc.scalar.activation(
                            pb[:, :], scs[i][:, :], EXP, scale=SCALE
                        )
                        prev.append((kb, pb))
                    if kb2 == 4 and pend is not None:
                        epi_pe(pend[0], pend[1], pend_outs)
                        pend = None

                def finish():
                    # drain attn@v for the last two k-blocks, then the
                    # DVE accumulator/denominator epilogue
                    for pkb, ppb in prev:
                        av_mms(pkb, ppb)
                    outs = []
                    for hh in range(2):
                        acc = accs[hh]
                        cpy = mi.tile([64, 512], F32, tag="cpy")
                        nc.vector.tensor_copy(cpy[:, :], acc[0:64, :])
                        den = mi.tile([1, 512], F32, tag="den")
                        nc.vector.tensor_copy(den[:, :], acc[64:65, :])
                        rec = mi.tile([1, 512], F32, tag="rec")
                        nc.vector.reciprocal_approx_fast(rec[:, :], den[:, :])
                        row = mi.tile([1, 512], BF16, tag="row")
                        nc.vector.tensor_copy(row[:, :], rec[:, :])
                        outs.append((cpy, row))
                    return outs

                return (hb, qc, finish)

            # ---- pre-phase ---------------------------------------------
            # dummy matmuls on the ones tile: ~3us of PE activity flips
            # the HAM clock gate to 8/8 before the first real unit, which
            # would otherwise run at 1.2 GHz while waiting out the window
            warm = ps2.tile([128, 512], F32, tag="psV", bufs=2, name="warm")
            for wi in range(22):
                nc.tensor.matmul(
                    warm[0:64, 0:64],
                    ones_sb[:, :],
                    ones_sb[:, :],
                    start=(wi == 0),
                    stop=(wi == 21),
                )
            # k/q ob0 for the first tokens
            kq_unit(0, 0, False)
            kq_unit(0, 0, True)

            # ---- phase 2: 12 passes -----------------------------------
            # pass 0 produces v pairs 0-1 just-in-time (block kb in step
            # kb, one step before attn@v needs it) and finishes pair-0
            # k/q as its DMAs land; later passes drain genq under quotas
            _P0_SPECIAL = {
                2: (0, 1, False),
                5: (0, 2, False),
                7: (0, 3, False),
                9: (0, 1, True),
            }

            def fill_p0(kb):
                v_unit(kb, 0, 256)
                if kb in _P0_SPECIAL:
                    kq_unit(*_P0_SPECIAL[kb])

            _QUOTA = [0, 11, 11, 7, 7, 7, 7, 7, 3, 2, 2, 0]

            def mk_fill(pi):
                quota = _QUOTA[pi]
                if quota == 0:
                    return None
                done = [0]

                def fill(kb):
                    want = (kb + 1) * quota // TB
                    while done[0] < want:
                        fill_gen()
                        done[0] += 1

                return fill

            # ---- phase 3: output projection (out^T form), emitted as
            # (ob, query-half) units so they pipeline through PSUM/DVE/DMA;
            # the qc=0 units for ob 0-3 run as pass-11 fillers (their last
            # attnT column block lands at pass 11's start via the deferred
            # epilogue)
            COPYFN = mybir.ActivationFunctionType.Identity

            def proj_unit(ob, qc, use_act=False):
                psp = ps2.tile(
                    [128, 512], F32, tag="psV", bufs=2, name=f"prj{ob}_{qc}"
                )
                for cb in range(CB):
                    nc.tensor.matmul(
                        psp[:, :],
                        wproj_sb[:, cb * C + ob * 128 : cb * C + (ob + 1) * 128],
                        attnT_sb[:, cb * NQ + qc * 512 : cb * NQ + (qc + 1) * 512],
                        start=(cb == 0),
                        stop=(cb == CB - 1),
                    )
                if qc == 0:
                    _ot_tiles[ob] = op_pool.tile(
                        [128, NQ], BF16, tag="out", bufs=6, name=f"ot{ob}"
                    )
                ot = _ot_tiles[ob]
                nc.vector.tensor_scalar_add(
                    ot[:, qc * 512 : (qc + 1) * 512],
                    psp[:, :],
                    bias_sb[:, ob : ob + 1],
                )
                nc.sync.dma_start(
                    out=outT[ob * 128 : (ob + 1) * 128, qc * 512 : (qc + 1) * 512],
                    in_=ot[:, qc * 512 : (qc + 1) * 512],
                )

            _ot_tiles = {}
            _P11_PROJ = {6: 0, 10: 1, 14: 2}

            def fill_p11(kb):
                if kb in _P11_PROJ:
                    proj_unit(_P11_PROJ[kb], 0)

            pend = emit_pass(0, 0, None, filler=fill_p0)
            for pi in range(1, 2 * HB):
                hb, qc = divmod(pi, 2)
                filler = fill_p11 if pi == 11 else mk_fill(pi)
                pend = emit_pass(hb, qc, pend, filler=filler)
            outs_last = pend[2]()
            for ob in range(3, CB):
                proj_unit(ob, 0, use_act=True)
            # qc=1 projection: only the cb5 term depends on the final
            # epilogue, so accumulate cb0-4 for all six obs first (the
            # retired score ring + accumulator banks hold the partials),
            # then emit the epilogue and finish each ob with one matmul
            q1p = {}
            bigs = [
                ps2.tile([128, NQ], F32, tag="psS", bufs=2, name=f"q1s{i}")
                for i in range(2)
            ]
            for ob in range(4):
                q1p[ob] = bigs[ob // 2][:, (ob % 2) * 512 : (ob % 2) * 512 + 512]
            for ob in (4, 5):
                q1p[ob] = ps2.tile(
                    [128, 512], F32, tag="psA", bufs=2, name=f"q1a{ob}"
                )[:, :]
            for ob in range(CB):
                for cb in range(CB - 1):
                    nc.tensor.matmul(
                        q1p[ob],
                        wproj_sb[:, cb * C + ob * 128 : cb * C + (ob + 1) * 128],
                        attnT_sb[:, cb * NQ + 512 : cb * NQ + NQ],
                        start=(cb == 0),
                        stop=False,
                    )
            epi_pe(pend[0], pend[1], outs_last)
            assert not genq

            wqxt[1].__exit__(None, None, None)
            wqxt[0].__exit__(None, None, None)

            for ob in range(CB):
                nc.tensor.matmul(
                    q1p[ob],
                    wproj_sb[:, 5 * C + ob * 128 : 5 * C + (ob + 1) * 128],
                    attnT_sb[:, 5 * NQ + 512 : 5 * NQ + NQ],
                    start=False,
                    stop=True,
                )
                ot = _ot_tiles[ob]
                nc.vector.tensor_scalar_add(
                    ot[:, 512:NQ], q1p[ob], bias_sb[:, ob : ob + 1]
                )
                nc.sync.dma_start(
                    out=outT[ob * 128 : (ob + 1) * 128, 512:NQ],
                    in_=ot[:, 512:NQ],
                )

    nc.finalize()
    return nc


_NC_CACHE = []


def _get_nc():
    if not _NC_CACHE:
        _NC_CACHE.append(_build())
    return _NC_CACHE[0]


def kernel(x, w_qkv, w_proj, b_proj):
    x = np.asarray(x, dtype=np.float32)
    w_qkv = np.asarray(w_qkv, dtype=np.float32)
    w_proj = np.asarray(w_proj, dtype=np.float32)
    b_proj = np.asarray(b_proj, dtype=np.float32)

    nc = _get_nc()

    wqkvT = w_qkv.T.astype(ml_dtypes.bfloat16)  # [C, 3C]
    wq3 = np.ascontiguousarray(wqkvT).reshape(CB, 128, 3 * C)  # [ci, p, o]
    wqp = np.concatenate(
        [
            wq3[:, :, o0 : o0 + w].transpose(1, 0, 2).reshape(128, CB * w)
            for o0, w in _WQ_GROUPS
        ],
        axis=1,
    )
    wqp = np.ascontiguousarray(wqp)
    # SBUF images: wproj cols = [ci][o], bias cols = [ci]
    wprojp = np.ascontiguousarray(
        w_proj.T.astype(ml_dtypes.bfloat16).reshape(CB, 128, C)
        .transpose(1, 0, 2)
        .reshape(128, CB * C)
    )
    biasp = np.ascontiguousarray(
        b_proj.astype(np.float32).reshape(CB, 128).T
    )

    in_maps = []
    for core in range(N_CORES):
        b, half = divmod(core, 2)
        # own 1024 query tokens first, then the other half (key order
        # within attention is permutation-invariant)
        mine = x[b, half * NQ : (half + 1) * NQ].T
        other = x[b, (1 - half) * NQ : (2 - half) * NQ].T
        xTc = np.concatenate([mine, other], axis=1).astype(ml_dtypes.bfloat16)
        # pack to the SBUF image: cols = [tch][ci][t]
        xTp = np.ascontiguousarray(
            xTc.reshape(CB, 128, 4, 512).transpose(1, 2, 0, 3).reshape(128, CB * N)
        )
        in_maps.append({"xTp": xTp, "wqp": wqp, "wprojp": wprojp, "biasp": biasp})

    res = run_bass_kernel_spmd(nc, in_maps, core_ids=list(range(N_CORES)))

    out = np.empty((B, N, C), dtype=np.float32)
    for core in range(N_CORES):
        b, half = divmod(core, 2)
        out[b, half * NQ : (half + 1) * NQ, :] = (
            res.results[core]["outT"].astype(np.float32).T
        )
    return out

